# revision 14
# baseline (speedup 1.0000x reference)
"""NeuroSAT message-passing kernel for 8 TRN2 NeuronCores (Bass/Tile).

Sharding: core k owns vars [375k, 375(k+1)) with BOTH polarities (750 lits:
slots 0..374 = positive lits, 375..749 = negated lits) and clauses
[1500k, 1500(k+1)). This makes the "flipped" polarity concat core-local.

Per round:
  MLP_L on local lits -> AllGather Lmsg (f32, node-major rows in DRAM)
  dma_gather incident cell rows -> one-hot segsum matmuls (transposed out)
  LSTM_C update -> MLP_C -> AllGather Cmsg -> gather -> segsum -> LSTM_L.

All states transposed [128 dim partitions, nodes free], f32 storage.
Matmul operands are bitcast to float32r (full-rate fp32 path for moving
free dim >= 256). Message-MLP output bias is folded in as a degree-weighted
rank-1 term at segsum eviction.
"""
import os
import sys

sys.path.insert(0, '/opt/trn_rl_repo')

import numpy as np
import ml_dtypes

import concourse.bass as bass
import concourse.bacc as bacc
import concourse.tile as tile
import concourse.mybir as mybir
from concourse import bass_utils

DIM = 128
R = 16
NV = 3000
NL = 6000
NC = 12000
NP = 50
W = 8
VB = NV // W          # 375 vars per core
VBP = VB + 1          # padded polarity block width (even fp32r matmul N)
LL = 2 * VBP          # 752 on-device lit slots per core (375 and 751 dead)
LC = NC // W          # 1500 clauses per core
LPAD = 768            # padded lit rows per rank in AG1 payload
CPAD = 1536           # padded clause rows per rank in AG2 payload
SEGW = 128            # segsum sub-tile width (bf16 matmuls)

N_SUB_C = (LC + SEGW - 1) // SEGW
N_SUB_L = (LL + SEGW - 1) // SEGW

F32 = mybir.dt.float32
F32R = mybir.dt.float32r
BF = mybir.dt.bfloat16
I16 = mybir.dt.int16

_cache = {}


def _wrap_idx(ix, n):
    """dma_gather index layout: logical i at [i%16, i//16], tiled x8 groups."""
    a = np.zeros((16, n // 16), np.int16)
    a[np.arange(len(ix)) % 16, np.arange(len(ix)) // 16] = ix
    return np.tile(a, (8, 1)).copy()


def _prep_side(owner_of_cell, owner_sub, owner_slot, gather_row, n_sub):
    """Build per-core gather idx + onehot + block schedule for one direction.

    owner_of_cell: core owning each cell's output node
    owner_sub:     sub-tile index (within core) of the output node
    owner_slot:    slot of output node within its sub-tile (0..SEGW-1)
    gather_row:    AG row to gather for each cell (source node)
    Returns (B[j] schedule, per-core idx arrays, per-core onehot arrays).
    """
    counts = np.zeros((W, n_sub), np.int64)
    for k in range(W):
        m = owner_of_cell == k
        counts[k] = np.bincount(owner_sub[m], minlength=n_sub)
    B = np.maximum(1, (counts.max(axis=0) + 127) // 128)  # blocks per sub-tile
    nblk = int(B.sum())
    idxs, ohs = [], []
    for k in range(W):
        idx = np.zeros(nblk * 128, np.int16)
        oh = np.zeros((128, nblk * SEGW), np.float32)
        bi = 0
        for j in range(n_sub):
            m = (owner_of_cell == k) & (owner_sub == j)
            rows = gather_row[m]
            slots = owner_slot[m]
            order = np.argsort(slots, kind='stable')
            rows, slots = rows[order], slots[order]
            n = len(rows)
            idx[bi * 128:bi * 128 + n] = rows
            pos = np.arange(n)
            oh[pos % 128, (bi + pos // 128) * SEGW + slots] = 1.0
            bi += int(B[j])
        idxs.append(_wrap_idx(idx, nblk * 128))
        ohs.append(oh.astype(ml_dtypes.bfloat16))
    return B, idxs, ohs


def _prep(lit_idx, clause_idx):
    l = np.asarray(lit_idx).astype(np.int64)
    c = np.asarray(clause_idx).astype(np.int64)
    key = np.unique(l * NC + c)
    l, c = key // NC, key % NC

    # lit n -> (rank, slot): rank by var block, slot offset +VB for negations
    v = l % NV
    pol = l // NV
    l_rank = v // VB
    l_slot = (v % VB) + VBP * pol
    c_rank = c // LC
    c_slot = c % LC

    ag1_row = l_rank * LPAD + l_slot        # Lmsg AG row of each cell's lit
    ag2_row = c_rank * CPAD + c_slot        # Cmsg AG row of each cell's clause

    # clause side (A^T @ Lmsg): output nodes = clauses, gather lits
    BC, idxC, ohC = _prep_side(c_rank, c_slot // SEGW, c_slot % SEGW, ag1_row,
                               N_SUB_C)
    # lit side (A @ Cmsg): output nodes = lits, gather clauses
    BL, idxL, ohL = _prep_side(l_rank, l_slot // SEGW, l_slot % SEGW, ag2_row,
                               N_SUB_L)

    # degrees for the folded MLP output bias
    degC = np.bincount(c, minlength=NC).astype(np.float32)
    degL = np.bincount(l, minlength=NL).astype(np.float32)
    degC_local = [degC[k * LC:(k + 1) * LC] for k in range(W)]
    degL_local = []
    for k in range(W):
        vlo, vhi = k * VB, (k + 1) * VB
        d = np.zeros(LL, np.float32)
        d[:VB] = degL[vlo:vhi]
        d[VBP:VBP + VB] = degL[NV + vlo:NV + vhi]
        degL_local.append(d)
    return dict(BC=BC, idxC=idxC, ohC=ohC, BL=BL, idxL=idxL, ohL=ohL,
                degC=degC_local, degL=degL_local)


def _chunks(total, width):
    out = []
    s = 0
    while s < total:
        out.append((s, min(width, total - s)))
        s += width
    return out


def _build(BC, BL):
    """Build the SPMD Bass program. BC/BL are block schedules (same all cores)."""
    NBC, NBL = int(sum(BC)), int(sum(BL))
    nc = bacc.Bacc("TRN2", target_bir_lowering=False, debug=False,
                   num_devices=W)

    def r(ap):
        return ap.bitcast(F32R)

    def din(name, shape, dt=F32):
        return nc.dram_tensor(name, shape, dt, kind="ExternalInput")

    h0L = din("h0L", [128, LL], F32R)
    h0C = din("h0C", [128, LC], F32R)
    wL1T = din("wL1T", [128, 128], F32R)
    wL2T = din("wL2T", [128, 128], F32R)
    wL3T = din("wL3T", [128, 128], BF)
    bL1 = din("bL1", [128, 1])
    bL2 = din("bL2", [128, 1])
    wC1T = din("wC1T", [128, 128], F32R)
    wC2T = din("wC2T", [128, 128], F32R)
    wC3T = din("wC3T", [128, 128], BF)
    bC1 = din("bC1", [128, 1])
    bC2 = din("bC2", [128, 1])
    B3C = din("B3C", [128, LC])
    B3L = din("B3L", [128, LL])
    wihCT = din("wihCT", [128, 512], F32R)
    whhCT = din("whhCT", [128, 512], F32R)
    bC4 = din("bC4", [128, 4])
    wihLAT = din("wihLAT", [128, 512], F32R)
    wihLBT = din("wihLBT", [128, 512], F32R)
    whhLT = din("whhLT", [128, 512], F32R)
    bL4 = din("bL4", [128, 4])
    wV1T = din("wV1T", [128, 128], F32R)
    wV2T = din("wV2T", [128, 128], F32R)
    wV3T = din("wV3T", [128, 1], F32R)
    bV1 = din("bV1", [128, 1])
    bV2 = din("bV2", [128, 1])
    idxC = din("idxC", [128, NBC * 8], I16)
    idxL = din("idxL", [128, NBL * 8], I16)
    ohC = din("ohC", [128, NBC * SEGW], BF)
    ohL = din("ohL", [128, NBL * SEGW], BF)

    votes_out = nc.dram_tensor("votes", [1, LL], F32, kind="ExternalOutput")

    Sig = mybir.ActivationFunctionType.Sigmoid
    Tanh = mybir.ActivationFunctionType.Tanh
    Relu = mybir.ActivationFunctionType.Relu

    with tile.TileContext(nc) as tc:
        with (
            tc.tile_pool(name="wp", bufs=1) as wp,
            tc.tile_pool(name="st", bufs=1) as st,
            tc.tile_pool(name="hp", bufs=2) as hp,
            tc.tile_pool(name="wk", bufs=1) as wk,
            tc.tile_pool(name="ps", bufs=1, space="PSUM") as ps,
            tc.tile_pool(name="dr", bufs=2, space="DRAM") as dr,
        ):
            def load(t, shape, dt=F32):
                sb = wp.tile(shape, dt, name=t.name + "_sb")
                nc.sync.dma_start(sb[:], t[:])
                return sb

            s_wL1T = load(wL1T, [128, 128], F32R)
            s_wL2T = load(wL2T, [128, 128], F32R)
            s_wL3T = load(wL3T, [128, 128], BF)
            s_bL1 = load(bL1, [128, 1])
            s_bL2 = load(bL2, [128, 1])
            s_wC1T = load(wC1T, [128, 128], F32R)
            s_wC2T = load(wC2T, [128, 128], F32R)
            s_wC3T = load(wC3T, [128, 128], BF)
            s_bC1 = load(bC1, [128, 1])
            s_bC2 = load(bC2, [128, 1])
            s_B3C = load(B3C, [128, LC])
            s_B3L = load(B3L, [128, LL])
            s_wihCT = load(wihCT, [128, 512], F32R)
            s_whhCT = load(whhCT, [128, 512], F32R)
            s_bC4 = load(bC4, [128, 4])
            s_wihLAT = load(wihLAT, [128, 512], F32R)
            s_wihLBT = load(wihLBT, [128, 512], F32R)
            s_whhLT = load(whhLT, [128, 512], F32R)
            s_bL4 = load(bL4, [128, 4])
            s_wV1T = load(wV1T, [128, 128], F32R)
            s_wV2T = load(wV2T, [128, 128], F32R)
            s_wV3T = load(wV3T, [128, 1], F32R)
            s_bV1 = load(bV1, [128, 1])
            s_bV2 = load(bV2, [128, 1])
            s_idxC = load(idxC, [128, NBC * 8], I16)
            s_idxL = load(idxL, [128, NBL * 8], I16)
            s_ohC = load(ohC, [128, NBC * SEGW], BF)
            s_ohL = load(ohL, [128, NBL * SEGW], BF)

            hL = hp.tile([128, LL], F32R, name="hL", tag="hL")
            nc.sync.dma_start(hL[:], h0L[:])
            hC = hp.tile([128, LC], F32R, name="hC", tag="hC")
            nc.sync.dma_start(hC[:], h0C[:])
            cL = st.tile([128, LL], F32, name="cL")
            nc.vector.memset(cL[:], 0.0)
            cC = st.tile([128, LC], F32, name="cC")
            nc.vector.memset(cC[:], 0.0)

            CH_C = _chunks(LC, 500)      # LSTM_C / MLP_C chunks (>=256)
            CH_L = _chunks(LL, VBP)      # LSTM_L / MLP_L chunks (376)

            def mlp2_T(x, w1, b1, w2, b2, chunks, n, nm, z2dt=F32R):
                """two Linear+ReLU layers in transposed layout."""
                z1 = wk.tile([128, n], F32R, name=f"{nm}_z1", tag=f"{nm}_z1")
                for c0, w in chunks:
                    p = ps.tile([128, 512], F32, name=f"{nm}_p1", tag="mlp",
                                bufs=2)
                    nc.tensor.matmul(p[:, :w], w1[:], x[:, c0:c0 + w],
                                     start=True, stop=True)
                    nc.scalar.activation(z1[:, c0:c0 + w], p[:, :w], Relu,
                                         bias=b1[:])
                z2 = wk.tile([128, n], z2dt, name=f"{nm}_z2", tag=f"{nm}_z2")
                for c0, w in chunks:
                    p = ps.tile([128, 512], F32, name=f"{nm}_p2", tag="mlp",
                                bufs=2)
                    nc.tensor.matmul(p[:, :w], w2[:], z1[:, c0:c0 + w],
                                     start=True, stop=True)
                    nc.scalar.activation(z2[:, c0:c0 + w], p[:, :w], Relu,
                                         bias=b2[:])
                return z2

            def msg_layer3(z2, w3, n, nm):
                """last MLP layer, node-major output [128, ntiles, 128]."""
                nt = (n + 127) // 128
                out = wk.tile([128, nt, 128], BF, name=f"{nm}_nm",
                              tag=f"{nm}_nm")
                if n % 128:
                    nc.vector.memset(out[:, nt - 1, :], 0.0)
                for t in range(nt):
                    m = min(128, n - t * 128)
                    p = ps.tile([128, 128], F32, name=f"{nm}_p3", tag="seg",
                                bufs=2)
                    nc.tensor.matmul(p[:m, :], z2[:, t * 128:t * 128 + m],
                                     w3[:], start=True, stop=True)
                    nc.vector.tensor_copy(out[:m, t, :], p[:m, :])
                return out

            def segsum(gath, oh, B, n, b3, nm):
                """one-hot segment-sum matmuls -> transposed [128, n]."""
                out = wk.tile([128, n], F32R, name=f"{nm}_seg", tag=f"{nm}_seg")
                bi = 0
                for j, bj in enumerate(B):
                    p = ps.tile([128, SEGW], F32, name=f"{nm}_ps", tag="seg",
                                bufs=2)
                    for t in range(int(bj)):
                        nc.tensor.matmul(
                            p[:], gath[:, bi, :],
                            oh[:, bi * SEGW:(bi + 1) * SEGW],
                            start=(t == 0), stop=(t == int(bj) - 1))
                        bi += 1
                    c0 = j * SEGW
                    w = min(SEGW, n - c0)
                    nc.vector.tensor_add(out[:, c0:c0 + w], p[:, :w],
                                         b3[:, c0:c0 + w])
                return out

            def lstm(xterms, whhT, h, c, bias4, chunks, n, nm):
                """xterms: list of (weightT[128,512], rhs_fn(c0,w)->AP).
                Returns new h tile; updates c in place."""
                hn = hp.tile([128, n], F32R, name=f"{nm}_h", tag=nm)
                for c0, w in chunks:
                    gps = []
                    for g in range(4):
                        p = ps.tile([128, 512], F32, name=f"{nm}_g{g}",
                                    tag="gates", bufs=4)
                        for ti, (wt, rf) in enumerate(xterms):
                            nc.tensor.matmul(p[:, :w],
                                             wt[:, g * 128:(g + 1) * 128],
                                             rf(c0, w),
                                             start=(ti == 0), stop=False)
                        nc.tensor.matmul(p[:, :w],
                                         whhT[:, g * 128:(g + 1) * 128],
                                         h[:, c0:c0 + w],
                                         start=False, stop=True)
                        gps.append(p)
                    iS = wk.tile([128, 512], F32, name=f"{nm}_i", tag="ls_i")
                    fS = wk.tile([128, 512], F32, name=f"{nm}_f", tag="ls_f")
                    gT = wk.tile([128, 512], F32, name=f"{nm}_gt", tag="ls_gt")
                    oS = wk.tile([128, 512], F32, name=f"{nm}_o", tag="ls_o")
                    nc.scalar.activation(iS[:, :w], gps[0][:, :w], Sig,
                                         bias=bias4[:, 0:1])
                    nc.scalar.activation(fS[:, :w], gps[1][:, :w], Sig,
                                         bias=bias4[:, 1:2])
                    nc.scalar.activation(gT[:, :w], gps[2][:, :w], Tanh,
                                         bias=bias4[:, 2:3])
                    nc.scalar.activation(oS[:, :w], gps[3][:, :w], Sig,
                                         bias=bias4[:, 3:4])
                    t1 = wk.tile([128, 512], F32, name=f"{nm}_t1", tag="ls_t1")
                    t2 = wk.tile([128, 512], F32, name=f"{nm}_t2", tag="ls_t2")
                    nc.vector.tensor_mul(t1[:, :w], fS[:, :w], c[:, c0:c0 + w])
                    nc.vector.tensor_mul(t2[:, :w], iS[:, :w], gT[:, :w])
                    nc.vector.tensor_add(c[:, c0:c0 + w], t1[:, :w], t2[:, :w])
                    th = wk.tile([128, 512], F32, name=f"{nm}_th", tag="ls_th")
                    nc.scalar.activation(th[:, :w], c[:, c0:c0 + w], Tanh)
                    nc.vector.tensor_mul(hn[:, c0:c0 + w], oS[:, :w],
                                         th[:, :w])
                return hn

            for rd in range(R):
                # ---- L messages ----
                zL2 = mlp2_T(hL, s_wL1T, s_bL1, s_wL2T, s_bL2, CH_L, LL, "mL",
                             z2dt=BF)
                LmsgNM = msg_layer3(zL2, s_wL3T, LL, "mL")
                ag1_in = dr.tile([LPAD, 128], BF, name="ag1i", tag="ag1i")
                nc.sync.dma_start(
                    ag1_in.rearrange("(b p) d -> p b d", p=128)[:, :6, :],
                    LmsgNM[:])
                ag1_out = dr.tile([W * LPAD, 128], BF, name="ag1o",
                                  tag="ag1o", addr_space="Shared")
                nc.gpsimd.collective_compute(
                    "AllGather", mybir.AluOpType.bypass,
                    replica_groups=[list(range(W))],
                    ins=[ag1_in.opt()], outs=[ag1_out.opt()])
                # ---- gather + segsum to clauses ----
                gathC = wk.tile([128, NBC, 128], BF, name="gathC",
                                tag="gathC", bufs=1)
                nc.gpsimd.dma_gather(
                    out_ap=gathC[:], in_ap=ag1_out[:], idxs_ap=s_idxC[:],
                    num_idxs=NBC * 128, num_idxs_reg=NBC * 128, elem_size=128,
                    single_packet=False)
                LCmsg = segsum(gathC, s_ohC, BC, LC, s_B3C, "sc")
                # ---- LSTM_C ----
                hC = lstm([(s_wihCT, lambda c0, w: LCmsg[:, c0:c0 + w])],
                          s_whhCT, hC, cC, s_bC4, CH_C, LC, "hC")
                # ---- C messages ----
                zC2 = mlp2_T(hC, s_wC1T, s_bC1, s_wC2T, s_bC2, CH_C, LC, "mC",
                             z2dt=BF)
                CmsgNM = msg_layer3(zC2, s_wC3T, LC, "mC")
                ag2_in = dr.tile([CPAD, 128], BF, name="ag2i", tag="ag2i")
                nc.sync.dma_start(
                    ag2_in.rearrange("(b p) d -> p b d", p=128)[:, :12, :],
                    CmsgNM[:])
                ag2_out = dr.tile([W * CPAD, 128], BF, name="ag2o",
                                  tag="ag2o", addr_space="Shared")
                nc.gpsimd.collective_compute(
                    "AllGather", mybir.AluOpType.bypass,
                    replica_groups=[list(range(W))],
                    ins=[ag2_in.opt()], outs=[ag2_out.opt()])
                # ---- gather + segsum to lits ----
                gathL = wk.tile([128, NBL, 128], BF, name="gathL",
                                tag="gathL", bufs=1)
                nc.gpsimd.dma_gather(
                    out_ap=gathL[:], in_ap=ag2_out[:], idxs_ap=s_idxL[:],
                    num_idxs=NBL * 128, num_idxs_reg=NBL * 128, elem_size=128,
                    single_packet=False)
                CLmsg = segsum(gathL, s_ohL, BL, LL, s_B3L, "sl")
                # ---- LSTM_L ----
                hLold = hL
                hL = lstm(
                    [(s_wihLAT, lambda c0, w: CLmsg[:, c0:c0 + w]),
                     (s_wihLBT,
                      lambda c0, w: hLold[:, (c0 + VBP) % LL:
                                          (c0 + VBP) % LL + w])],
                    s_whhLT, hLold, cL, s_bL4, CH_L, LL, "hL")

            # ---- vote MLP ----
            zV2 = mlp2_T(hL, s_wV1T, s_bV1, s_wV2T, s_bV2, CH_L, LL, "mV")
            votes_sb = wk.tile([1, LL], F32, name="votes_sb", tag="votes")
            for c0, w in CH_L:
                p = ps.tile([1, 512], F32, name="vp", tag="gates", bufs=4)
                nc.tensor.matmul(p[:, :w], s_wV3T[:], zV2[:, c0:c0 + w],
                                 start=True, stop=True)
                nc.vector.tensor_copy(votes_sb[:, c0:c0 + w], p[:, :w])
            nc.sync.dma_start(votes_out[:], votes_sb[:])

    nc.compile()
    return nc


def _get_program(BC, BL):
    key = (tuple(int(b) for b in BC), tuple(int(b) for b in BL))
    if key not in _cache:
        _cache[key] = _build(key[0], key[1])
    return _cache[key]


def kernel(lit_idx, clause_idx, n_vars, n_probs, params):
    n_vars = int(n_vars)
    n_probs = int(n_probs)
    assert n_vars == NV and n_probs == NP

    g = _prep(lit_idx, clause_idx)
    nc = _get_program(g["BC"], g["BL"])

    p = {k: {kk: np.asarray(vv, np.float32) for kk, vv in v.items()}
         if isinstance(v, dict) else np.asarray(v, np.float32)
         for k, v in params.items()}

    def f(x):
        return np.ascontiguousarray(x, dtype=np.float32)

    Lm, Cm, Lu, Cu, Vo = (p['L_msg'], p['C_msg'], p['L_update'],
                          p['C_update'], p['L_vote'])

    hL0 = np.repeat((p['L_init_w'][:, 0] + p['L_init_b'])[:, None], LL, 1)
    hC0 = np.repeat((p['C_init_w'][:, 0] + p['C_init_b'])[:, None], LC, 1)

    base = {
        "h0L": f(hL0), "h0C": f(hC0),
        "wL1T": f(Lm['w1'].T), "wL2T": f(Lm['w2'].T), "wL3T": np.ascontiguousarray(Lm['w3'].T).astype(ml_dtypes.bfloat16),
        "bL1": f(Lm['b1'][:, None]), "bL2": f(Lm['b2'][:, None]),
        "wC1T": f(Cm['w1'].T), "wC2T": f(Cm['w2'].T), "wC3T": np.ascontiguousarray(Cm['w3'].T).astype(ml_dtypes.bfloat16),
        "bC1": f(Cm['b1'][:, None]), "bC2": f(Cm['b2'][:, None]),
        "wihCT": f(Cu['wih'].T), "whhCT": f(Cu['whh'].T),
        "bC4": f((Cu['bih'] + Cu['bhh']).reshape(4, 128).T),
        "wihLAT": f(Lu['wih'][:, :128].T), "wihLBT": f(Lu['wih'][:, 128:].T),
        "whhLT": f(Lu['whh'].T),
        "bL4": f((Lu['bih'] + Lu['bhh']).reshape(4, 128).T),
        "wV1T": f(Vo['w1'].T), "wV2T": f(Vo['w2'].T), "wV3T": f(Vo['w3'].T),
        "bV1": f(Vo['b1'][:, None]), "bV2": f(Vo['b2'][:, None]),
    }

    in_maps = []
    for k in range(W):
        m = dict(base)
        m["B3C"] = f(np.outer(Lm['b3'], g["degC"][k]))
        m["B3L"] = f(np.outer(Cm['b3'], g["degL"][k]))
        m["idxC"] = g["idxC"][k]
        m["idxL"] = g["idxL"][k]
        m["ohC"] = g["ohC"][k]
        m["ohL"] = g["ohL"][k]
        in_maps.append(m)

    res = bass_utils.run_bass_kernel_spmd(
        nc, in_maps, core_ids=list(range(W)),
        trace=bool(os.environ.get("NEUROSAT_TRACE")))
    kernel.last_results = res

    votes = np.zeros(NL, np.float32)
    vb3 = float(Vo['b3'][0])
    for k in range(W):
        vk = res.results[k]["votes"][0] + vb3
        vlo = k * VB
        votes[vlo:vlo + VB] = vk[:VB]
        votes[NV + vlo:NV + vlo + VB] = vk[VBP:VBP + VB]

    vote_join = np.stack([votes[:NV], votes[NV:]], axis=1)  # [NV, 2]
    return vote_join.reshape(n_probs, -1).mean(axis=1).astype(np.float32)


# revision 15
# speedup vs baseline: 1.0517x; 1.0517x over previous
"""NeuroSAT message-passing kernel for 8 TRN2 NeuronCores (Bass/Tile).

Sharding: core k owns vars [375k, 375(k+1)) with BOTH polarities (750 lits:
slots 0..374 = positive lits, 375..749 = negated lits) and clauses
[1500k, 1500(k+1)). This makes the "flipped" polarity concat core-local.

Per round:
  MLP_L on local lits -> AllGather Lmsg (f32, node-major rows in DRAM)
  dma_gather incident cell rows -> one-hot segsum matmuls (transposed out)
  LSTM_C update -> MLP_C -> AllGather Cmsg -> gather -> segsum -> LSTM_L.

All states transposed [128 dim partitions, nodes free], f32 storage.
Matmul operands are bitcast to float32r (full-rate fp32 path for moving
free dim >= 256). Message-MLP output bias is folded in as a degree-weighted
rank-1 term at segsum eviction.
"""
import os
import sys

sys.path.insert(0, '/opt/trn_rl_repo')

import numpy as np
import ml_dtypes

import concourse.bass as bass
import concourse.bacc as bacc
import concourse.tile as tile
import concourse.mybir as mybir
from concourse import bass_utils

DIM = 128
R = 16
NV = 3000
NL = 6000
NC = 12000
NP = 50
W = 8
VB = NV // W          # 375 vars per core
VBP = VB + 1          # padded polarity block width (even fp32r matmul N)
LL = 2 * VBP          # 752 on-device lit slots per core (375 and 751 dead)
LC = NC // W          # 1500 clauses per core
LPAD = LL             # lit rows per rank in AG1 payload (752)
CPAD = LC             # clause rows per rank in AG2 payload (1500)
SEGW = 128            # segsum sub-tile width (bf16 matmuls)

N_SUB_C = (LC + SEGW - 1) // SEGW
N_SUB_L = (LL + SEGW - 1) // SEGW

F32 = mybir.dt.float32
F32R = mybir.dt.float32r
BF = mybir.dt.bfloat16
I16 = mybir.dt.int16

_cache = {}


def _wrap_idx(ix, n):
    """dma_gather index layout: logical i at [i%16, i//16], tiled x8 groups."""
    a = np.zeros((16, n // 16), np.int16)
    a[np.arange(len(ix)) % 16, np.arange(len(ix)) // 16] = ix
    return np.tile(a, (8, 1)).copy()


def _prep_side(owner_of_cell, owner_sub, owner_slot, gather_row, n_sub):
    """Build per-core gather idx + onehot + block schedule for one direction.

    owner_of_cell: core owning each cell's output node
    owner_sub:     sub-tile index (within core) of the output node
    owner_slot:    slot of output node within its sub-tile (0..SEGW-1)
    gather_row:    AG row to gather for each cell (source node)
    Returns (B[j] schedule, per-core idx arrays, per-core onehot arrays).
    """
    counts = np.zeros((W, n_sub), np.int64)
    for k in range(W):
        m = owner_of_cell == k
        counts[k] = np.bincount(owner_sub[m], minlength=n_sub)
    B = np.maximum(1, (counts.max(axis=0) + 127) // 128)  # blocks per sub-tile
    nblk = int(B.sum())
    idxs, ohs = [], []
    for k in range(W):
        idx = np.zeros(nblk * 128, np.int16)
        oh = np.zeros((128, nblk * SEGW), np.float32)
        bi = 0
        for j in range(n_sub):
            m = (owner_of_cell == k) & (owner_sub == j)
            rows = gather_row[m]
            slots = owner_slot[m]
            order = np.argsort(slots, kind='stable')
            rows, slots = rows[order], slots[order]
            n = len(rows)
            idx[bi * 128:bi * 128 + n] = rows
            pos = np.arange(n)
            oh[pos % 128, (bi + pos // 128) * SEGW + slots] = 1.0
            bi += int(B[j])
        idxs.append(_wrap_idx(idx, nblk * 128))
        ohs.append(oh.astype(ml_dtypes.bfloat16))
    return B, idxs, ohs


def _prep(lit_idx, clause_idx):
    l = np.asarray(lit_idx).astype(np.int64)
    c = np.asarray(clause_idx).astype(np.int64)
    key = np.unique(l * NC + c)
    l, c = key // NC, key % NC

    # lit n -> (rank, slot): rank by var block, slot offset +VB for negations
    v = l % NV
    pol = l // NV
    l_rank = v // VB
    l_slot = (v % VB) + VBP * pol
    c_rank = c // LC
    c_slot = c % LC

    ag1_row = l_rank * LPAD + l_slot        # Lmsg AG row of each cell's lit
    ag2_row = c_rank * CPAD + c_slot        # Cmsg AG row of each cell's clause

    # clause side (A^T @ Lmsg): output nodes = clauses, gather lits
    BC, idxC, ohC = _prep_side(c_rank, c_slot // SEGW, c_slot % SEGW, ag1_row,
                               N_SUB_C)
    # lit side (A @ Cmsg): output nodes = lits, gather clauses
    BL, idxL, ohL = _prep_side(l_rank, l_slot // SEGW, l_slot % SEGW, ag2_row,
                               N_SUB_L)

    # degrees for the folded MLP output bias
    degC = np.bincount(c, minlength=NC).astype(np.float32)
    degL = np.bincount(l, minlength=NL).astype(np.float32)
    degC_local = [degC[k * LC:(k + 1) * LC] for k in range(W)]
    degL_local = []
    for k in range(W):
        vlo, vhi = k * VB, (k + 1) * VB
        d = np.zeros(LL, np.float32)
        d[:VB] = degL[vlo:vhi]
        d[VBP:VBP + VB] = degL[NV + vlo:NV + vhi]
        degL_local.append(d)
    return dict(BC=BC, idxC=idxC, ohC=ohC, BL=BL, idxL=idxL, ohL=ohL,
                degC=degC_local, degL=degL_local)


def _chunks(total, width):
    out = []
    s = 0
    while s < total:
        out.append((s, min(width, total - s)))
        s += width
    return out


def _build(BC, BL):
    """Build the SPMD Bass program. BC/BL are block schedules (same all cores)."""
    NBC, NBL = int(sum(BC)), int(sum(BL))
    nc = bacc.Bacc("TRN2", target_bir_lowering=False, debug=False,
                   num_devices=W)

    def r(ap):
        return ap.bitcast(F32R)

    def din(name, shape, dt=F32):
        return nc.dram_tensor(name, shape, dt, kind="ExternalInput")

    h0L = din("h0L", [128, LL], F32R)
    h0C = din("h0C", [128, LC], F32R)
    wL1T = din("wL1T", [128, 128], F32R)
    wL2T = din("wL2T", [128, 128], F32R)
    wL3T = din("wL3T", [128, 128], BF)
    bL1 = din("bL1", [128, 1])
    bL2 = din("bL2", [128, 1])
    wC1T = din("wC1T", [128, 128], F32R)
    wC2T = din("wC2T", [128, 128], F32R)
    wC3T = din("wC3T", [128, 128], BF)
    bC1 = din("bC1", [128, 1])
    bC2 = din("bC2", [128, 1])
    B3C = din("B3C", [128, LC])
    B3L = din("B3L", [128, LL])
    wihCT = din("wihCT", [128, 512], F32R)
    whhCT = din("whhCT", [128, 512], F32R)
    bC4 = din("bC4", [128, 4])
    wihLAT = din("wihLAT", [128, 512], F32R)
    wihLBT = din("wihLBT", [128, 512], F32R)
    whhLT = din("whhLT", [128, 512], F32R)
    bL4 = din("bL4", [128, 4])
    wV1T = din("wV1T", [128, 128], F32R)
    wV2T = din("wV2T", [128, 128], F32R)
    wV3T = din("wV3T", [128, 1], F32R)
    bV1 = din("bV1", [128, 1])
    bV2 = din("bV2", [128, 1])
    idxC = din("idxC", [128, NBC * 8], I16)
    idxL = din("idxL", [128, NBL * 8], I16)
    ohC = din("ohC", [128, NBC * SEGW], BF)
    ohL = din("ohL", [128, NBL * SEGW], BF)

    votes_out = nc.dram_tensor("votes", [1, LL], F32, kind="ExternalOutput")

    Sig = mybir.ActivationFunctionType.Sigmoid
    Tanh = mybir.ActivationFunctionType.Tanh
    Relu = mybir.ActivationFunctionType.Relu

    with tile.TileContext(nc) as tc:
        with (
            tc.tile_pool(name="wp", bufs=1) as wp,
            tc.tile_pool(name="st", bufs=1) as st,
            tc.tile_pool(name="hp", bufs=2) as hp,
            tc.tile_pool(name="wk", bufs=1) as wk,
            tc.tile_pool(name="ps", bufs=1, space="PSUM") as ps,
            tc.tile_pool(name="dr", bufs=2, space="DRAM") as dr,
        ):
            def load(t, shape, dt=F32):
                sb = wp.tile(shape, dt, name=t.name + "_sb")
                nc.sync.dma_start(sb[:], t[:])
                return sb

            s_wL1T = load(wL1T, [128, 128], F32R)
            s_wL2T = load(wL2T, [128, 128], F32R)
            s_wL3T = load(wL3T, [128, 128], BF)
            s_bL1 = load(bL1, [128, 1])
            s_bL2 = load(bL2, [128, 1])
            s_wC1T = load(wC1T, [128, 128], F32R)
            s_wC2T = load(wC2T, [128, 128], F32R)
            s_wC3T = load(wC3T, [128, 128], BF)
            s_bC1 = load(bC1, [128, 1])
            s_bC2 = load(bC2, [128, 1])
            s_B3C = load(B3C, [128, LC])
            s_B3L = load(B3L, [128, LL])
            s_wihCT = load(wihCT, [128, 512], F32R)
            s_whhCT = load(whhCT, [128, 512], F32R)
            s_bC4 = load(bC4, [128, 4])
            s_wihLAT = load(wihLAT, [128, 512], F32R)
            s_wihLBT = load(wihLBT, [128, 512], F32R)
            s_whhLT = load(whhLT, [128, 512], F32R)
            s_bL4 = load(bL4, [128, 4])
            s_wV1T = load(wV1T, [128, 128], F32R)
            s_wV2T = load(wV2T, [128, 128], F32R)
            s_wV3T = load(wV3T, [128, 1], F32R)
            s_bV1 = load(bV1, [128, 1])
            s_bV2 = load(bV2, [128, 1])
            s_idxC = load(idxC, [128, NBC * 8], I16)
            s_idxL = load(idxL, [128, NBL * 8], I16)
            s_ohC = load(ohC, [128, NBC * SEGW], BF)
            s_ohL = load(ohL, [128, NBL * SEGW], BF)

            hL = hp.tile([128, LL], F32R, name="hL", tag="hL")
            nc.sync.dma_start(hL[:], h0L[:])
            hC = hp.tile([128, LC], F32R, name="hC", tag="hC")
            nc.sync.dma_start(hC[:], h0C[:])
            cL = st.tile([128, LL], F32, name="cL")
            nc.vector.memset(cL[:], 0.0)
            cC = st.tile([128, LC], F32, name="cC")
            nc.vector.memset(cC[:], 0.0)

            CH_C = _chunks(LC, 500)      # LSTM_C / MLP_C chunks (>=256)
            CH_L = _chunks(LL, VBP)      # LSTM_L / MLP_L chunks (376)

            def mlp2_T(x, w1, b1, w2, b2, chunks, n, nm, z2dt=F32R):
                """two Linear+ReLU layers in transposed layout."""
                z1 = wk.tile([128, n], F32R, name=f"{nm}_z1", tag=f"{nm}_z1")
                for c0, w in chunks:
                    p = ps.tile([128, 512], F32, name=f"{nm}_p1", tag="mlp",
                                bufs=2)
                    nc.tensor.matmul(p[:, :w], w1[:], x[:, c0:c0 + w],
                                     start=True, stop=True)
                    nc.scalar.activation(z1[:, c0:c0 + w], p[:, :w], Relu,
                                         bias=b1[:])
                z2 = wk.tile([128, n], z2dt, name=f"{nm}_z2", tag=f"{nm}_z2")
                for c0, w in chunks:
                    p = ps.tile([128, 512], F32, name=f"{nm}_p2", tag="mlp",
                                bufs=2)
                    nc.tensor.matmul(p[:, :w], w2[:], z1[:, c0:c0 + w],
                                     start=True, stop=True)
                    nc.scalar.activation(z2[:, c0:c0 + w], p[:, :w], Relu,
                                         bias=b2[:])
                return z2

            def msg_layer3(z2, w3, n, nm):
                """last MLP layer, node-major output [128, ntiles, 128]."""
                nt = (n + 127) // 128
                out = wk.tile([128, nt, 128], BF, name=f"{nm}_nm",
                              tag=f"{nm}_nm")
                if n % 128:
                    nc.vector.memset(out[:, nt - 1, :], 0.0)
                for t in range(nt):
                    m = min(128, n - t * 128)
                    p = ps.tile([128, 128], F32, name=f"{nm}_p3", tag="seg",
                                bufs=2)
                    nc.tensor.matmul(p[:m, :], z2[:, t * 128:t * 128 + m],
                                     w3[:], start=True, stop=True)
                    nc.vector.tensor_copy(out[:m, t, :], p[:m, :])
                return out

            def segsum(gath, oh, B, n, b3, nm):
                """one-hot segment-sum matmuls -> transposed [128, n]."""
                out = wk.tile([128, n], F32R, name=f"{nm}_seg", tag=f"{nm}_seg")
                bi = 0
                for j, bj in enumerate(B):
                    p = ps.tile([128, SEGW], F32, name=f"{nm}_ps", tag="seg",
                                bufs=2)
                    for t in range(int(bj)):
                        nc.tensor.matmul(
                            p[:], gath[:, bi, :],
                            oh[:, bi * SEGW:(bi + 1) * SEGW],
                            start=(t == 0), stop=(t == int(bj) - 1))
                        bi += 1
                    c0 = j * SEGW
                    w = min(SEGW, n - c0)
                    nc.vector.tensor_add(out[:, c0:c0 + w], p[:, :w],
                                         b3[:, c0:c0 + w])
                return out

            def lstm(xterms, whhT, h, c, bias4, chunks, n, nm):
                """xterms: list of (weightT[128,512], rhs_fn(c0,w)->AP).
                Returns new h tile; updates c in place."""
                hn = hp.tile([128, n], F32R, name=f"{nm}_h", tag=nm)
                for c0, w in chunks:
                    gps = []
                    for g in range(4):
                        p = ps.tile([128, 512], F32, name=f"{nm}_g{g}",
                                    tag="gates", bufs=4)
                        for ti, (wt, rf) in enumerate(xterms):
                            nc.tensor.matmul(p[:, :w],
                                             wt[:, g * 128:(g + 1) * 128],
                                             rf(c0, w),
                                             start=(ti == 0), stop=False)
                        nc.tensor.matmul(p[:, :w],
                                         whhT[:, g * 128:(g + 1) * 128],
                                         h[:, c0:c0 + w],
                                         start=False, stop=True)
                        gps.append(p)
                    iS = wk.tile([128, 512], F32, name=f"{nm}_i", tag="ls_i")
                    fS = wk.tile([128, 512], F32, name=f"{nm}_f", tag="ls_f")
                    gT = wk.tile([128, 512], F32, name=f"{nm}_gt", tag="ls_gt")
                    oS = wk.tile([128, 512], F32, name=f"{nm}_o", tag="ls_o")
                    nc.scalar.activation(iS[:, :w], gps[0][:, :w], Sig,
                                         bias=bias4[:, 0:1])
                    nc.scalar.activation(fS[:, :w], gps[1][:, :w], Sig,
                                         bias=bias4[:, 1:2])
                    nc.scalar.activation(gT[:, :w], gps[2][:, :w], Tanh,
                                         bias=bias4[:, 2:3])
                    nc.scalar.activation(oS[:, :w], gps[3][:, :w], Sig,
                                         bias=bias4[:, 3:4])
                    t1 = wk.tile([128, 512], F32, name=f"{nm}_t1", tag="ls_t1")
                    t2 = wk.tile([128, 512], F32, name=f"{nm}_t2", tag="ls_t2")
                    nc.vector.tensor_mul(t1[:, :w], fS[:, :w], c[:, c0:c0 + w])
                    nc.vector.tensor_mul(t2[:, :w], iS[:, :w], gT[:, :w])
                    nc.vector.tensor_add(c[:, c0:c0 + w], t1[:, :w], t2[:, :w])
                    th = wk.tile([128, 512], F32, name=f"{nm}_th", tag="ls_th")
                    nc.scalar.activation(th[:, :w], c[:, c0:c0 + w], Tanh)
                    nc.vector.tensor_mul(hn[:, c0:c0 + w], oS[:, :w],
                                         th[:, :w])
                return hn

            for rd in range(R):
                # ---- L messages ----
                zL2 = mlp2_T(hL, s_wL1T, s_bL1, s_wL2T, s_bL2, CH_L, LL, "mL",
                             z2dt=BF)
                LmsgNM = msg_layer3(zL2, s_wL3T, LL, "mL")
                ag1_in = dr.tile([LPAD, 128], BF, name="ag1i", tag="ag1i")
                nc.sync.dma_start(
                    ag1_in[:640].rearrange("(b p) d -> p b d", p=128)[:],
                    LmsgNM[:, :5, :])
                nc.sync.dma_start(
                    ag1_in[640:].rearrange("(b p) d -> p b d", p=112)[:],
                    LmsgNM[:112, 5, :])
                ag1_out = dr.tile([W * LPAD, 128], BF, name="ag1o",
                                  tag="ag1o", addr_space="Shared")
                nc.gpsimd.collective_compute(
                    "AllGather", mybir.AluOpType.bypass,
                    replica_groups=[list(range(W))],
                    ins=[ag1_in.opt()], outs=[ag1_out.opt()])
                # ---- gather + segsum to clauses ----
                gathC = wk.tile([128, NBC, 128], BF, name="gathC",
                                tag="gathC", bufs=1)
                nc.gpsimd.dma_gather(
                    out_ap=gathC[:], in_ap=ag1_out[:], idxs_ap=s_idxC[:],
                    num_idxs=NBC * 128, num_idxs_reg=NBC * 128, elem_size=128,
                    single_packet=False)
                LCmsg = segsum(gathC, s_ohC, BC, LC, s_B3C, "sc")
                # ---- LSTM_C ----
                hC = lstm([(s_wihCT, lambda c0, w: LCmsg[:, c0:c0 + w])],
                          s_whhCT, hC, cC, s_bC4, CH_C, LC, "hC")
                # ---- C messages ----
                zC2 = mlp2_T(hC, s_wC1T, s_bC1, s_wC2T, s_bC2, CH_C, LC, "mC",
                             z2dt=BF)
                CmsgNM = msg_layer3(zC2, s_wC3T, LC, "mC")
                ag2_in = dr.tile([CPAD, 128], BF, name="ag2i", tag="ag2i")
                nc.sync.dma_start(
                    ag2_in[:1408].rearrange("(b p) d -> p b d", p=128)[:],
                    CmsgNM[:, :11, :])
                nc.sync.dma_start(
                    ag2_in[1408:].rearrange("(b p) d -> p b d", p=92)[:],
                    CmsgNM[:92, 11, :])
                ag2_out = dr.tile([W * CPAD, 128], BF, name="ag2o",
                                  tag="ag2o", addr_space="Shared")
                nc.gpsimd.collective_compute(
                    "AllGather", mybir.AluOpType.bypass,
                    replica_groups=[list(range(W))],
                    ins=[ag2_in.opt()], outs=[ag2_out.opt()])
                # ---- gather + segsum to lits ----
                gathL = wk.tile([128, NBL, 128], BF, name="gathL",
                                tag="gathL", bufs=1)
                nc.gpsimd.dma_gather(
                    out_ap=gathL[:], in_ap=ag2_out[:], idxs_ap=s_idxL[:],
                    num_idxs=NBL * 128, num_idxs_reg=NBL * 128, elem_size=128,
                    single_packet=False)
                CLmsg = segsum(gathL, s_ohL, BL, LL, s_B3L, "sl")
                # ---- LSTM_L ----
                hLold = hL
                hL = lstm(
                    [(s_wihLAT, lambda c0, w: CLmsg[:, c0:c0 + w]),
                     (s_wihLBT,
                      lambda c0, w: hLold[:, (c0 + VBP) % LL:
                                          (c0 + VBP) % LL + w])],
                    s_whhLT, hLold, cL, s_bL4, CH_L, LL, "hL")

            # ---- vote MLP ----
            zV2 = mlp2_T(hL, s_wV1T, s_bV1, s_wV2T, s_bV2, CH_L, LL, "mV")
            votes_sb = wk.tile([1, LL], F32, name="votes_sb", tag="votes")
            for c0, w in CH_L:
                p = ps.tile([1, 512], F32, name="vp", tag="gates", bufs=4)
                nc.tensor.matmul(p[:, :w], s_wV3T[:], zV2[:, c0:c0 + w],
                                 start=True, stop=True)
                nc.vector.tensor_copy(votes_sb[:, c0:c0 + w], p[:, :w])
            nc.sync.dma_start(votes_out[:], votes_sb[:])

    nc.compile()
    return nc


def _get_program(BC, BL):
    key = (tuple(int(b) for b in BC), tuple(int(b) for b in BL))
    if key not in _cache:
        _cache[key] = _build(key[0], key[1])
    return _cache[key]


def kernel(lit_idx, clause_idx, n_vars, n_probs, params):
    n_vars = int(n_vars)
    n_probs = int(n_probs)
    assert n_vars == NV and n_probs == NP

    g = _prep(lit_idx, clause_idx)
    nc = _get_program(g["BC"], g["BL"])

    p = {k: {kk: np.asarray(vv, np.float32) for kk, vv in v.items()}
         if isinstance(v, dict) else np.asarray(v, np.float32)
         for k, v in params.items()}

    def f(x):
        return np.ascontiguousarray(x, dtype=np.float32)

    Lm, Cm, Lu, Cu, Vo = (p['L_msg'], p['C_msg'], p['L_update'],
                          p['C_update'], p['L_vote'])

    hL0 = np.repeat((p['L_init_w'][:, 0] + p['L_init_b'])[:, None], LL, 1)
    hC0 = np.repeat((p['C_init_w'][:, 0] + p['C_init_b'])[:, None], LC, 1)

    base = {
        "h0L": f(hL0), "h0C": f(hC0),
        "wL1T": f(Lm['w1'].T), "wL2T": f(Lm['w2'].T), "wL3T": np.ascontiguousarray(Lm['w3'].T).astype(ml_dtypes.bfloat16),
        "bL1": f(Lm['b1'][:, None]), "bL2": f(Lm['b2'][:, None]),
        "wC1T": f(Cm['w1'].T), "wC2T": f(Cm['w2'].T), "wC3T": np.ascontiguousarray(Cm['w3'].T).astype(ml_dtypes.bfloat16),
        "bC1": f(Cm['b1'][:, None]), "bC2": f(Cm['b2'][:, None]),
        "wihCT": f(Cu['wih'].T), "whhCT": f(Cu['whh'].T),
        "bC4": f((Cu['bih'] + Cu['bhh']).reshape(4, 128).T),
        "wihLAT": f(Lu['wih'][:, :128].T), "wihLBT": f(Lu['wih'][:, 128:].T),
        "whhLT": f(Lu['whh'].T),
        "bL4": f((Lu['bih'] + Lu['bhh']).reshape(4, 128).T),
        "wV1T": f(Vo['w1'].T), "wV2T": f(Vo['w2'].T), "wV3T": f(Vo['w3'].T),
        "bV1": f(Vo['b1'][:, None]), "bV2": f(Vo['b2'][:, None]),
    }

    in_maps = []
    for k in range(W):
        m = dict(base)
        m["B3C"] = f(np.outer(Lm['b3'], g["degC"][k]))
        m["B3L"] = f(np.outer(Cm['b3'], g["degL"][k]))
        m["idxC"] = g["idxC"][k]
        m["idxL"] = g["idxL"][k]
        m["ohC"] = g["ohC"][k]
        m["ohL"] = g["ohL"][k]
        in_maps.append(m)

    res = bass_utils.run_bass_kernel_spmd(
        nc, in_maps, core_ids=list(range(W)),
        trace=bool(os.environ.get("NEUROSAT_TRACE")))
    kernel.last_results = res

    votes = np.zeros(NL, np.float32)
    vb3 = float(Vo['b3'][0])
    for k in range(W):
        vk = res.results[k]["votes"][0] + vb3
        vlo = k * VB
        votes[vlo:vlo + VB] = vk[:VB]
        votes[NV + vlo:NV + vlo + VB] = vk[VBP:VBP + VB]

    vote_join = np.stack([votes[:NV], votes[NV:]], axis=1)  # [NV, 2]
    return vote_join.reshape(n_probs, -1).mean(axis=1).astype(np.float32)
